# revision 31
# baseline (speedup 1.0000x reference)
"""Performer (FAVOR+) attention TRN2 Bass kernel (v3).

Problem: B=4, N=4096, D=1024, H=16, HD=64, M=256 random features.
Sharding: 8 cores = (batch b = c//2) x (sequence half s = c%2).
Each core handles all 16 heads for 2048 query tokens and 2048 k/v
tokens; partial kv/ksum is AllReduced over seq-half pairs (split into
two collectives, heads 0:8 / 8:16, overlapped with compute).

Math simplifications (exact):
  - q-side max-over-features subtraction skipped (cancels in the
    numerator/denominator ratio; magnitudes stay well inside fp32).
  - max(normalizer, 1e-6) clamp skipped (normalizer >> 1e-6 here).
  - k-side diag term exp(CDIAG*||kh||^2 + EXP_BIAS) multiplied into
    the k features (edpe), so one bias-free exp covers both heads of a
    pair via a block-diagonal projection matrix.
  - q-side diag & 1/sqrt(M) cancel in the out/normalizer ratio.
  - bv, bo folded host-side: out += bv @ Wo.T + bo.

v3 changes vs v2:
  - q features via the same block-diagonal projection as the k side:
    K=128 matmuls instead of K=64 (which measured ~2x slower).
  - ksum folded into the q-feature exp as a per-partition ln(ksum)
    bias; kv pre-divided by ksum (kvn).  The out matmul then uses an
    augmented lhsT [kvn | ones] so one [128,512] matmul yields both
    the 64 output dims AND the normalizer replicated on the other 64
    partitions.  Normalizer reciprocal on DVE (reciprocal_approx_fast,
    ~18 bits; run over all 128 partitions since the op mis-lowers at
    partition offset 64 - only the valid norm half is read) instead of
    ACT Ln+Exp; a small SBUF->SBUF DMA bounce aligns rec partitions
    with out partitions.
  - v_nat ones column memset on DVE instead of a 32K-descriptor DMA
    (was ~50us of startup serialization).
  - wq + first two q chunks prefetched (right-side SBUF heap, ACT-queue
    triggers so they fire after 1v, off the startup flood); 1q's first
    token chunk hoisted into stage B's ACT-bound window.
  - k' features for pairs 0-1 precomputed in 1v's tail (idle ACT),
    shrinking stage B's ACT-bound window.
  - q/k projection path in bf16 (inputs + weights); collective payload
    in bf16.  Startup DMA flood ~5MB vs ~18MB originally.

Precision: q/k path bf16 inputs with fp32 PSUM accumulation, khT/qhT
staged fp32r; v / k' / kvn / q' / o_proj path bf16.
"""

import math
import sys

import numpy as np
import ml_dtypes

for _p in ("/opt/trn_rl_repo",):
    if _p not in sys.path:
        sys.path.insert(0, _p)

from concourse import bass, tile, mybir
from concourse.bass_utils import run_bass_kernel_spmd

F32 = mybir.dt.float32
F32R = mybir.dt.float32r
BF16 = mybir.dt.bfloat16

B, N, D = 4, 4096, 1024
H, HD, M = 16, 64, 256
NS = 2048  # tokens per core

SNORM = float(HD) ** -0.25
CDIAG = -0.5 * SNORM * SNORM  # -0.0625
EXP_BIAS = -0.5 * math.log(float(M))


def _build():
    nc = bass.Bass(trn_type="TRN2", target_bir_lowering=False, num_devices=8)

    qt = nc.dram_tensor("qt", [128, 8, NS], BF16, kind="ExternalInput")
    kt = nc.dram_tensor("kt", [128, 8, NS], BF16, kind="ExternalInput")
    vt = nc.dram_tensor("vt", [128, 8, NS], BF16, kind="ExternalInput")
    wqt = nc.dram_tensor("wqt", [128, 8, D], BF16, kind="ExternalInput")
    wkt = nc.dram_tensor("wkt", [128, 8, D], BF16, kind="ExternalInput")
    wvt = nc.dram_tensor("wvt", [128, 8, D], BF16, kind="ExternalInput")
    wot = nc.dram_tensor("wot", [128, 8, D], BF16, kind="ExternalInput")
    pjbdt = nc.dram_tensor("pjbdt", [128, 2 * M], F32R, kind="ExternalInput")
    mskc = nc.dram_tensor("mskc", [128, 2], F32R, kind="ExternalInput")
    bqc = nc.dram_tensor("bqc", [128, 8], F32, kind="ExternalInput")
    bkc = nc.dram_tensor("bkc", [128, 8], F32, kind="ExternalInput")
    out = nc.dram_tensor("out", [NS, D], F32, kind="ExternalOutput")

    mult = mybir.AluOpType.mult
    Exp = mybir.ActivationFunctionType.Exp
    Ln = mybir.ActivationFunctionType.Ln

    with tile.TileContext(nc) as tc:
        _q2 = [nc.sync, nc.scalar]

        # ---------------- persistent constants ----------------
        pjbd, free_pjbd = tc.tile([128, 2 * M], F32R, name="pjbd")  # block-diag
        msk2, free_msk2 = tc.tile([128, 2], F32R, name="msk2")
        bq_sb, free_bq = tc.tile([128, 8], F32, name="bq_sb")
        bk_sb, free_bk = tc.tile([128, 8], F32, name="bk_sb")
        ebias, free_ebias = tc.tile([128, 1], F32, name="ebias")

        # ---------------- big staging buffers ----------------
        # qhT for token blocks tc4=0,1, produced early: 1q's first two
        # chunks are hoisted into stage B where the PE has slack (and the
        # extra work keeps its clock ramped).  Lives left of khT so it
        # survives the post-B frees (LIFO).
        qtmp, free_qtmp = tc.tile([128, 8, 1024], F32R, name="qtmp")
        # khT[p, dc, t] = kh[t, dc*128+p]  (head h=2*dc+(p>=64))
        khT, free_khT = tc.tile([128, 8, NS], F32R, name="khT")
        # v_nat[p, tcc, h, 0:64] = vh[tcc*128+p, h*64+d]; col 64 = 1.0
        v_nat, free_v_nat = tc.tile([128, 16, 16, 65], BF16, name="v_nat")
        # edpe[p, pair, tcc, hh] = exp(CDIAG*||kh||^2 + EXP_BIAS)
        edpe, free_edpe = tc.tile([128, 8, 16, 2], F32, name="edpe")
        # k' features for pair 0, precomputed in 1v's tail where the
        # ACT engine is idle; stage B then only runs its kv matmuls.
        k2p0, free_k2p0 = tc.tile([128, 16, 512], BF16, name="k2p0")

        # v-proj weights live on the right-side heap: their DMA must not
        # inherit a WAR dependency on wk_sb's SBUF range (that cost an
        # 8us PE gap between phases 1k and 1v).
        wv_sb, free_wv = tc.tile([128, 8, D], BF16, name="wv_sb", side="right")

        # ---------------- phase 1k: khT = Wk @ k.T + bk ----------------
        # wk and the first k chunk are SPLIT into ic-halves held in
        # separate tiles: the first 32 matmuls (K-half ic 0:4) depend on
        # only 1.5MB of DMA, so the PE starts ~15us earlier; the second
        # K-half accumulates into the same PSUM groups when its data
        # lands.  Chunk 0 therefore needs all 8 PSUM banks at once.
        with tc.tile_pool(name="wk", bufs=1) as wkpool, \
             tc.tile_pool(name="k0", bufs=1) as k0pool, \
             tc.tile_pool(name="kin", bufs=2) as kinpool:
            # tiny constants first, then the critical lo-half slices
            nc.sync.dma_start(pjbd[:, :], pjbdt[:, :])
            nc.scalar.dma_start(msk2[:, :], mskc[:, :])
            nc.sync.dma_start(bk_sb[:, :], bkc[:, :])
            nc.scalar.dma_start(bq_sb[:, :], bqc[:, :])
            nc.vector.memset(ebias[:, :], EXP_BIAS)
            nc.vector.memset(v_nat[:, :, :, 64:65], 1.0)
            wk_lo = wkpool.tile([128, 4, D], BF16, name="wk_lo")
            wk_hi = wkpool.tile([128, 4, D], BF16, name="wk_hi")
            kt0_lo = k0pool.tile([128, 4, 512], BF16, name="kt0_lo")
            kt0_hi = k0pool.tile([128, 4, 512], BF16, name="kt0_hi")
            for _ic in range(4):
                _q2[_ic % 2].dma_start(wk_lo[:, _ic, :], wkt[:, _ic, :])
            nc.sync.dma_start(kt0_lo[:, :, :], kt[:, 0:4, 0:512])
            for _ic in range(4):
                _q2[_ic % 2].dma_start(wk_hi[:, _ic, :], wkt[:, 4 + _ic, :])
            nc.scalar.dma_start(kt0_hi[:, :, :], kt[:, 4:8, 0:512])

            def wk_sl(ic, dc):
                t = wk_lo if ic < 4 else wk_hi
                return t[:, ic % 4, dc * 128:(dc + 1) * 128]

            with tc.tile_pool(name="pk0", bufs=8, space="PSUM") as pk0pool:
                ps0 = [pk0pool.tile([128, 512], F32, name="pk0")
                       for _ in range(8)]
                for half, kth in ((0, kt0_lo), (1, kt0_hi)):
                    for dc in range(8):
                        for icx in range(4):
                            ic = half * 4 + icx
                            nc.tensor.matmul(
                                ps0[dc][:, :],
                                wk_sl(ic, dc),
                                kth[:, icx, :],
                                start=(ic == 0), stop=(ic == 7),
                            )
                        if half == 1:
                            nc.scalar.add(khT[:, dc, 0:512],
                                          ps0[dc][:, :], bk_sb[:, dc:dc + 1])

            with tc.tile_pool(name="pk", bufs=4, space="PSUM") as pkpool:
              for tcc in range(1, 4):
                kt_in = kinpool.tile([128, 8, 512], BF16, name="kt_in")
                # chunk 1 on sync (immediate); 2/3 on the ACT queue so
                # they fire only after earlier bias-adds, off the startup
                # flood but still ~15us ahead of their consumers.
                _q2[0 if tcc < 2 else 1].dma_start(
                    kt_in[:, :, :], kt[:, :, tcc * 512:(tcc + 1) * 512])
                for dc in range(8):
                    ps = pkpool.tile([128, 512], F32, name="pk")
                    for ic in range(8):
                        nc.tensor.matmul(
                            ps[:, :],
                            wk_sl(ic, dc),
                            kt_in[:, ic, :],
                            start=(ic == 0), stop=(ic == 7),
                        )
                    # bias add on ACT (idle in this phase); DVE does squares.
                    nc.scalar.add(khT[:, dc, tcc * 512:(tcc + 1) * 512],
                                  ps[:, :], bk_sb[:, dc:dc + 1])
                if tcc == 1:
                    # wv triggers on the ACT queue: fire only after tcc=0's
                    # bias-adds execute (~t=65us), keeping the 2MB off the
                    # critical startup DMA flood; done well before 1v needs
                    # them.
                    for _ic in range(8):
                        nc.scalar.dma_start(wv_sb[:, _ic, :], wvt[:, _ic, :])

        # ---------------- phase 1v + A2 interleaved ----------------
        # A2: dpe[p,pair,tcc,hh] = CDIAG*||kh||^2 (via masked matmul), then
        # edpe = exp(dpe + EXP_BIAS) in one activation.
        with tc.tile_pool(name="vin", bufs=3) as vinpool, \
             tc.tile_pool(name="sq", bufs=3) as sqpool, \
             tc.tile_pool(name="pdp", bufs=1, space="PSUM") as pdppool, \
             tc.tile_pool(name="kfe", bufs=2, space="PSUM") as kfepool, \
             tc.tile_pool(name="pv", bufs=3, space="PSUM") as pvpool:
            dps = pdppool.tile([128, 8, 16, 2], F32, name="dps")
            for tcc in range(16):
                if tcc % 2 == 0:
                    # 256-token loads: 512B descriptors (half the count of
                    # per-128-token loads, which starved the 1v matmuls)
                    vt_in2 = vinpool.tile([128, 8, 256], BF16, name="vt_in2")
                    _q2[(tcc // 2 + 1) % 2].dma_start(
                        vt_in2[:, :, :], vt[:, :, tcc * 128:(tcc + 2) * 128])
                vsl = slice((tcc % 2) * 128, (tcc % 2) * 128 + 128)
                for dc in range(2):
                    ps = pvpool.tile([128, 8, 64], F32, name="pv")
                    for ic in range(8):
                        nc.tensor.matmul(
                            ps[:, :, :],
                            vt_in2[:, ic, vsl],
                            wv_sb[:, ic, dc * 512:(dc + 1) * 512],
                            start=(ic == 0), stop=(ic == 7),
                        )
                    nc.vector.tensor_copy(v_nat[:, tcc, dc * 8:(dc + 1) * 8, 0:64],
                                          ps[:, :, :])
                if tcc % 2 == 0:
                    pair = tcc // 2
                    for t4 in range(4):
                        srcp = khT[:, pair, t4 * 512:(t4 + 1) * 512]
                        sq = sqpool.tile([128, 512], F32R, name="sq")
                        nc.vector.tensor_tensor(sq[:, :], srcp, srcp, mult)
                        for c in range(4):
                            nc.tensor.matmul(
                                dps[:, pair, t4 * 4 + c, :],
                                sq[:, c * 128:(c + 1) * 128],
                                msk2[:, :],
                                start=True, stop=True)
                    if pair < 1:
                        # early edpe for pair 0 (its k' is built below)
                        nc.scalar.activation(edpe[:, pair, :, :],
                                             dps[:, pair, :, :], Exp,
                                             bias=ebias[:, 0:1])
                # 1v tail: k-feature units for pair 0 (2 per iteration),
                # using the otherwise-idle ACT engine.
                if tcc >= 8:
                    for u in range(2):
                        ut = (tcc - 8) * 2 + u
                        kfe = kfepool.tile([128, 512], F32, name="kfe")
                        nc.tensor.matmul(kfe[:, :],
                                         khT[:, 0, ut * 128:(ut + 1) * 128],
                                         pjbd[:, :], start=True, stop=True)
                        nc.scalar.activation(k2p0[:, ut, :],
                                             kfe[:, :], Exp)
                        nc.vector.tensor_scalar_mul(
                            k2p0[:, ut, 0:256], k2p0[:, ut, 0:256],
                            edpe[:, 0, ut, 0:1])
                        nc.vector.tensor_scalar_mul(
                            k2p0[:, ut, 256:512], k2p0[:, ut, 256:512],
                            edpe[:, 0, ut, 1:2])
            nc.scalar.activation(edpe[:, 1:8, :, :], dps[:, 1:8, :, :], Exp,
                                 bias=ebias[:, 0:1])

        # ---------------- prefetch wq + first q chunks (right heap) -----
        # Issued before stage B so phase 1q's first matmul isn't gated on
        # DMA after the collectives block the gpsimd queue.  All triggers
        # ride the ACT queue: ACT only reaches them after 1v's activations
        # (~t=200us), so the 8MB doesn't crowd the startup DMA flood, yet
        # lands long before 1q starts.
        wq_sb, free_wq = tc.tile([128, 8, D], BF16, name="wq_sb", side="right")
        qt_in0, free_qt0 = tc.tile([128, 8, 512], BF16, name="qt_in0",
                                   side="right")
        qt_in1, free_qt1 = tc.tile([128, 8, 512], BF16, name="qt_in1",
                                   side="right")
        for _ic in range(8):
            nc.scalar.dma_start(wq_sb[:, _ic, :], wqt[:, _ic, :])
        nc.scalar.dma_start(qt_in0[:, :, :], qt[:, :, 0:512])
        nc.scalar.dma_start(qt_in1[:, :, :], qt[:, :, 512:1024])

        # ---------------- collectives DRAM staging (bf16 payload) -------
        with tc.tile_pool(name="dramb", bufs=4, space="DRAM") as dramb:
            cin1 = dramb.tile([128, 8, 2, 65], BF16, name="cin1")
            cout1 = dramb.tile([128, 8, 2, 65], BF16, name="cout1")
            cin2 = dramb.tile([128, 8, 2, 65], BF16, name="cin2")
            cout2 = dramb.tile([128, 8, 2, 65], BF16, name="cout2")
            kvA, free_kvA = tc.tile([128, 8, 2, 65], BF16, name="kvA")
            kvB, free_kvB = tc.tile([128, 8, 2, 65], BF16, name="kvB")

            # ---------------- stage B: k features + kvT accumulation -----
            # kf[t, 0:256 | 256:512] = (khT_pair)^T @ [pj|0 ; 0|pj]
            # k2 = exp(kf) * edpe (bf16); kvT[m, hd65] = sum_t k2 v_nat
            # pq opens outside stage B so the hoisted 1q chunk and the
            # post-B 1q loop share it: 3 + 3 + 2 = 8 PSUM banks.
            stageE_ctx = tc.tile_pool(name="pq", bufs=3, space="PSUM")
            pqpool = stageE_ctx.__enter__()
            with tc.tile_pool(name="pkf", bufs=3, space="PSUM") as pkfpool, \
                 tc.tile_pool(name="k2", bufs=3) as k2pool, \
                 tc.tile_pool(name="pkv", bufs=2, space="PSUM") as pkvpool:
                for pair in range(8):
                    kvp = pkvpool.tile([128, 2, 2, 65], F32, name="kvp")
                    for tcc in range(16):
                        if pair < 1:
                            k2 = k2p0[:, tcc, :]
                        else:
                            kf = pkfpool.tile([128, 512], F32, name="kf")
                            nc.tensor.matmul(
                                kf[:, :],
                                khT[:, pair, tcc * 128:(tcc + 1) * 128],
                                pjbd[:, :], start=True, stop=True)
                            k2 = k2pool.tile([128, 512], BF16, name="k2")
                            nc.scalar.activation(k2[:, :], kf[:, :], Exp)
                            nc.vector.tensor_scalar_mul(
                                k2[:, 0:256], k2[:, 0:256],
                                edpe[:, pair, tcc, 0:1])
                            nc.vector.tensor_scalar_mul(
                                k2[:, 256:512], k2[:, 256:512],
                                edpe[:, pair, tcc, 1:2])
                        # kvp's 4 slices share one PSUM bank = one zero
                        # region: a single accumulation group. start marks
                        # the whole region pending-zero, so only the first
                        # matmul starts and only the last stops.
                        for hh in range(2):
                            h = 2 * pair + hh
                            for fc in range(2):
                                nc.tensor.matmul(
                                    kvp[:, hh, fc, :],
                                    k2[:, hh * 256 + fc * 128:hh * 256 + (fc + 1) * 128],
                                    v_nat[:, tcc, h, 0:65],
                                    start=(tcc == 0 and hh == 0 and fc == 0),
                                    stop=(tcc == 15 and hh == 1 and fc == 1),
                                )
                    if pair < 4:
                        nc.vector.tensor_copy(kvA[:, 2 * (pair % 4):2 * (pair % 4) + 2, :, :],
                                              kvp[:, :, :, :])
                    else:
                        nc.vector.tensor_copy(kvB[:, 2 * (pair % 4):2 * (pair % 4) + 2, :, :],
                                              kvp[:, :, :, :])
                    if pair == 3:
                        nc.gpsimd.dma_start(cin1[:, :, :, :], kvA[:, :, :, :])
                        nc.gpsimd.collective_compute(
                            "AllReduce", mybir.AluOpType.add,
                            replica_groups=[[0, 1], [2, 3], [4, 5], [6, 7]],
                            ins=[cin1.opt()], outs=[cout1.opt()],
                        )
                    if pair in (3, 5):
                        # hoisted 1q token chunks 0 (pair 3) and 1 (pair 5):
                        # stage B is ACT-bound, so the PE has slack here and
                        # the extra matmuls keep its clock ramped.  Bias-add
                        # on DVE to keep ACT clear; results go to qtmp (read
                        # by stage E as the tc4=0,1 feature source).
                        hq = 0 if pair == 3 else 1
                        hq_in = qt_in0 if hq == 0 else qt_in1
                        for dc in range(8):
                            ps = pqpool.tile([128, 512], F32, name="pq")
                            for ic in range(8):
                                nc.tensor.matmul(
                                    ps[:, :],
                                    wq_sb[:, ic, dc * 128:(dc + 1) * 128],
                                    hq_in[:, ic, :],
                                    start=(ic == 0), stop=(ic == 7),
                                )
                            nc.vector.tensor_scalar_add(
                                qtmp[:, dc, hq * 512:(hq + 1) * 512],
                                ps[:, :], bq_sb[:, dc:dc + 1])
                    if pair == 7:
                        nc.gpsimd.dma_start(cin2[:, :, :, :], kvB[:, :, :, :])
                        nc.gpsimd.collective_compute(
                            "AllReduce", mybir.AluOpType.add,
                            replica_groups=[[0, 1], [2, 3], [4, 5], [6, 7]],
                            ins=[cin2.opt()], outs=[cout2.opt()],
                        )

            # khT / v_nat / edpe no longer needed; reuse for qhT (LIFO order).
            free_kvB()
            free_kvA()
            free_k2p0()
            free_edpe()
            free_v_nat()
            free_khT()

            # ---------------- phase 1q: qhT = Wq @ q.T + bq (overlaps cc) --
            qhT, free_qhT = tc.tile([128, 8, NS], F32R, name="qhT")
            # kv gathered from the collectives, bf16: [feat, head, fc, 0:65]
            kvbf_a, free_kvbf_a = tc.tile([128, 8, 2, 65], BF16,
                                           name="kvbf_a")
            kvbf_b, free_kvbf_b = tc.tile([128, 8, 2, 65], BF16,
                                          name="kvbf_b")
            # ksum column widened to f32 (reciprocal requires f32 input)
            ksumf_a, free_ksumf_a = tc.tile([128, 8, 2, 1], F32,
                                            name="ksumf_a")
            ksumf_b, free_ksumf_b = tc.tile([128, 8, 2, 1], F32,
                                            name="ksumf_b")
            # kvo: augmented bf16 lhsT per (head, fc): even heads
            # [kvn | ones], odd heads [ones | kvn] so out lands on the
            # head's attn partition half and norm on the other half.
            kvo_a, free_kvo_a = tc.tile([128, 8, 2, 128], BF16, name="kvo_a")
            kvo_b, free_kvo_b = tc.tile([128, 8, 2, 128], BF16, name="kvo_b")
            rk_a, free_rk_a = tc.tile([128, 8, 2, 1], F32, name="rk_a")
            rk_b, free_rk_b = tc.tile([128, 8, 2, 1], F32, name="rk_b")
            lnk_a, free_lnk_a = tc.tile([128, 8, 2, 1], F32, name="lnk_a")
            lnk_b, free_lnk_b = tc.tile([128, 8, 2, 1], F32, name="lnk_b")
            if True:
                qts = [qt_in0, qt_in1]
                for tcc in range(2, 4):
                    if tcc >= 2:
                        _q2[tcc % 2].dma_start(
                            qts[tcc % 2][:, :, :],
                            qt[:, :, tcc * 512:(tcc + 1) * 512])
                    qt_in = qts[tcc % 2]
                    for dc in range(8):
                        ps = pqpool.tile([128, 512], F32, name="pq")
                        for ic in range(8):
                            nc.tensor.matmul(
                                ps[:, :],
                                wq_sb[:, ic, dc * 128:(dc + 1) * 128],
                                qt_in[:, ic, :],
                                start=(ic == 0), stop=(ic == 7),
                            )
                        nc.scalar.add(qhT[:, dc, tcc * 512:(tcc + 1) * 512],
                                      ps[:, :], bq_sb[:, dc:dc + 1])

                # ------------ kv gather + kvn/lnk prep (hides under 1q) --
                # per-collective halves: pairs 0-3 (heads 0-7) only
                # depend on cc1, so stage E can start before cc2 lands.
                for half, (kvbf, ksumf, rk, lnk, cout) in enumerate(
                        ((kvbf_a, ksumf_a, rk_a, lnk_a, cout1),
                         (kvbf_b, ksumf_b, rk_b, lnk_b, cout2))):
                    nc.sync.dma_start(kvbf[:, :, :, :], cout[:, :, :, :])
                    nc.vector.tensor_copy(ksumf[:, :, :, :],
                                          kvbf[:, :, :, 64:65])
                    for hx in range(8):
                        nc.vector.reciprocal(rk[:, hx, :, :],
                                             ksumf[:, hx, :, :])
                    nc.scalar.activation(lnk[:, :, :, :], ksumf[:, :, :, :],
                                         Ln)
                    kvo = kvo_a if half == 0 else kvo_b
                    for hx in range(8):
                        h = half * 8 + hx
                        off_ones = 64 if h % 2 == 0 else 0
                        off_kv = 0 if h % 2 == 0 else 64
                        nc.vector.memset(
                            kvo[:, hx, :, off_ones:off_ones + 64], 1.0)
                        for fc in range(2):
                            nc.vector.tensor_scalar_mul(
                                kvo[:, hx, fc, off_kv:off_kv + 64],
                                kvbf[:, hx, fc, 0:64], rk[:, hx, fc, 0:1])

            stageE_ctx.__exit__(None, None, None)
            free_qt1()
            free_qt0()
            free_wq()
            free_wv()

            # ---------------- stage E: q features, attention, o_proj --
            with tc.tile_pool(name="wo", bufs=1) as wopool, \
                 tc.tile_pool(name="attn", bufs=2) as attnpool, \
                 tc.tile_pool(name="qp", bufs=4) as qppool, \
                 tc.tile_pool(name="rec", bufs=6) as recpool, \
                 tc.tile_pool(name="osb", bufs=2) as osbpool, \
                 tc.tile_pool(name="pqf", bufs=2, space="PSUM") as pqfpool, \
                 tc.tile_pool(name="oaug", bufs=4, space="PSUM") as oaugpool, \
                 tc.tile_pool(name="po5", bufs=2, space="PSUM") as po5pool:
                    wo_sb = wopool.tile([128, 8, D], BF16, name="wo_sb")
                    for _ic in range(8):
                        nc.sync.dma_start(wo_sb[:, _ic, :], wot[:, _ic, :])

                    def emit_oproj(src_tc4, attn_t, tcc, j):
                        # one o_proj output block [128 tokens x 512 dout]
                        p5 = po5pool.tile([128, 512], F32, name="p5")
                        for pair in range(8):
                            nc.tensor.matmul(
                                p5[:, :],
                                attn_t[:, pair, tcc * 128:(tcc + 1) * 128],
                                wo_sb[:, pair, j * 512:(j + 1) * 512],
                                start=(pair == 0), stop=(pair == 7),
                            )
                        o_sb = osbpool.tile([128, 512], F32, name="o_sb")
                        nc.scalar.copy(o_sb[:, :], p5[:, :])
                        nc.sync.dma_start(
                            out[src_tc4 * 512 + tcc * 128:
                                src_tc4 * 512 + (tcc + 1) * 128,
                                j * 512:(j + 1) * 512],
                            o_sb[:, :])

                    prev_attn = None
                    for tc4 in range(4):
                        tsl = slice(tc4 * 512, (tc4 + 1) * 512)
                        qsrc = qtmp if tc4 < 2 else qhT
                        attn = attnpool.tile([128, 8, 512], BF16, name="attn")
                        for pair in range(8):
                            oaug = [oaugpool.tile([128, 512], F32, name="oa")
                                    for _ in range(2)]
                            for c in range(4):
                                hh, fc = c // 2, c % 2
                                h = 2 * pair + hh
                                qf = pqfpool.tile([128, 512], F32, name="qf")
                                nc.tensor.matmul(
                                    qf[:, :],
                                    pjbd[:, c * 128:(c + 1) * 128],
                                    qtmp[:, pair, tsl] if tc4 < 2
                                    else qhT[:, pair, tsl],
                                    start=True, stop=True)
                                qp = qppool.tile([128, 512], BF16, name="qp")
                                lnk_h = lnk_a if h < 8 else lnk_b
                                kvo_h = kvo_a if h < 8 else kvo_b
                                nc.scalar.activation(
                                    qp[:, :], qf[:, :], Exp,
                                    bias=lnk_h[:, h % 8, fc, 0:1])
                                nc.tensor.matmul(
                                    oaug[hh][:, :],
                                    kvo_h[:, h % 8, fc, :],
                                    qp[:, :],
                                    start=(fc == 0), stop=(fc == 1))
                            # even head: out rows 0:64, norm rows 64:128;
                            # odd head: norm rows 0:64, out rows 64:128.
                            # reciprocal_approx_fast mis-lowers at partition
                            # offset 64, so run it over all 128 partitions
                            # (offset 0) and only read the valid norm half —
                            # the other half is 1/out garbage, never used.
                            rec = recpool.tile([128, 512], F32, name="rec")
                            rec2 = recpool.tile([128, 512], F32, name="rec2")
                            nc.vector.reciprocal_approx_fast(
                                rec[:, :], oaug[0][:, :])
                            nc.vector.reciprocal_approx_fast(
                                rec2[:, :], oaug[1][:, :])
                            # swap halves so rec partitions line up with out
                            recb = recpool.tile([128, 512], F32, name="recb")
                            nc.sync.dma_start(recb[0:64, :], rec[64:128, :])
                            nc.sync.dma_start(recb[64:128, :], rec2[0:64, :])
                            nc.vector.tensor_tensor(
                                attn[0:64, pair, :], oaug[0][0:64, :],
                                recb[0:64, :], mult)
                            nc.vector.tensor_tensor(
                                attn[64:128, pair, :], oaug[1][64:128, :],
                                recb[64:128, :], mult)
                            # software-pipeline o_proj: interleave the
                            # previous tc4's 8 output blocks between this
                            # tc4's 8 pair groups so its PE time hides
                            # under the ACT/DVE-bound feature work.
                            if prev_attn is not None:
                                emit_oproj(tc4 - 1, prev_attn,
                                           pair // 2, pair % 2)
                        prev_attn = attn
                    for tcc in range(4):
                        for j in range(2):
                            emit_oproj(3, prev_attn, tcc, j)

            for f in (free_lnk_b, free_lnk_a, free_rk_b, free_rk_a,
                      free_kvo_b, free_kvo_a, free_ksumf_b, free_ksumf_a,
                      free_kvbf_b, free_kvbf_a, free_qhT):
                f()

        free_qtmp()

        for f in (free_ebias, free_bk, free_bq, free_msk2, free_pjbd):
            f()

    # TRN2 walrus codegen allows at most 1 sync wait per instruction
    # (2 on InstEventSemaphore); split excess waits into event semaphores.
    import bass_rust
    bass_rust.generate_event_semaphores(nc)
    # custom-DVE ops (reciprocal_approx_fast) are InstISA subclasses whose
    # .instr bytes are only populated by this pass; without it walrus
    # codegen fails with "ISA wrong length".
    from concourse.library_overlay import lower_extended_insts
    lower_extended_insts(nc)
    return nc


_CACHE = {}


def _get_nc():
    if "nc" not in _CACHE:
        _CACHE["nc"] = _build()
    return _CACHE["nc"]


def _shard(x, dtype=np.float32):
    # [2048, 1024] token-slice -> [128, 8, 2048] with [p, ic, t] = x[t, ic*128+p]
    return np.ascontiguousarray(
        x.T.reshape(8, 128, NS).transpose(1, 0, 2)).astype(dtype)


def _wlayout(w, dtype=np.float32):
    # W [D, D] -> [128, 8, D] with [p, ic, d] = W[d, ic*128+p]
    return np.ascontiguousarray(
        w.T.reshape(8, 128, D).transpose(1, 0, 2)).astype(dtype)


def _run(nc, in_maps, trace=False, tmpdir=None):
    return run_bass_kernel_spmd(nc, in_maps, list(range(8)), trace=trace,
                                tmpdir=tmpdir)


def _host_inputs(q, k, v, Wq, bq, Wk, bk, Wv, bv, Wo, bo, proj):
    bf16 = ml_dtypes.bfloat16

    pjs = (proj.T * SNORM).astype(np.float32)          # [64, 256]
    pjbdt = np.zeros((128, 2 * M), dtype=np.float32)   # block-diag
    pjbdt[0:64, 0:M] = pjs
    pjbdt[64:128, M:2 * M] = pjs
    wqt = _wlayout(Wq, bf16)
    wkt = _wlayout(Wk, bf16)
    wvt = _wlayout(Wv, bf16)
    wot = _wlayout(Wo, bf16)
    bqc = np.ascontiguousarray(bq.reshape(8, 128).T).astype(np.float32)
    bkc = np.ascontiguousarray(bk.reshape(8, 128).T).astype(np.float32)
    mskc = np.zeros((128, 2), dtype=np.float32)
    mskc[0:64, 0] = CDIAG
    mskc[64:128, 1] = CDIAG

    in_maps = []
    for c in range(8):
        b, s = divmod(c, 2)
        sl = slice(s * NS, (s + 1) * NS)
        in_maps.append({
            "qt": _shard(q[b, sl, :], bf16),
            "kt": _shard(k[b, sl, :], bf16),
            "vt": _shard(v[b, sl, :], bf16),
            "wqt": wqt, "wkt": wkt, "wvt": wvt, "wot": wot,
            "pjbdt": pjbdt, "bqc": bqc, "bkc": bkc,
            "mskc": mskc,
        })
    return in_maps


def kernel(q, k, v, Wq, bq, Wk, bk, Wv, bv, Wo, bo, proj,
           _trace=False, _tmpdir=None):
    nc = _get_nc()
    in_maps = _host_inputs(q, k, v, Wq, bq, Wk, bk, Wv, bv, Wo, bo, proj)

    res = _run(nc, in_maps, trace=_trace, tmpdir=_tmpdir)

    bo_eff = (bv @ Wo.T + bo).astype(np.float32)
    full = np.empty((B, N, D), dtype=np.float32)
    for c in range(8):
        b, s = divmod(c, 2)
        full[b, s * NS:(s + 1) * NS, :] = res.results[c]["out"] + bo_eff

    if _trace:
        return full, res
    return full
